# revision 23
# baseline (speedup 1.0000x reference)
"""Trainium2 Bass kernel for nn_ConvNet: char-CNN + word-CNN encoder.

reference semantics (B=32, L=256, C=16, D=128, kernel 3, padding 1):
  char path: chr_emb = chr_table[words_in_char]        [B,L,C,D]
             word_conv = conv1d(chr_emb, W_chr) + b    over C
             char_feats = word_conv.max(axis=C)        [B,L,D]
  word path: word_emb = word_table[word_vector]        [B,L,D]
             out = conv1d(word_emb, W_word) + b        over L
  output: stack([out, char_feats.T]) -> [2, B, D, L] float32

Strategy (8 cores, data-parallel over B, 4 sentences/core):
  * char path avoids the 64MB embedding gather:
      UT_k = chr_table @ W_k.T  (host precompute, bf16, char bias folded
      into the tap-1 table), then per char position
      y[:, c] = UT_1[:,idx[c]] + UT_0[:,idx[c-1]] + UT_2[:,idx[c+1]]
    realized as one-hot matmuls: the padded index rows (period-17 layout,
    -1 pads) are DMA-broadcast to all 128 partitions in bf16 up-front
    (8 large chunks), one-hots built by a single is_equal vs a per-core
    iota column (DVE / Pool) or an ABS+RELU pair (ACT), then 3 shifted
    bf16 matmuls per 32-word tile accumulate the conv in one PSUM bank
    and DVE max-reduces over the 16 char positions.
  * word path (fp32/fp32r, runs first, fills the PE ramp-up window):
    one fused 1024-row indirect-DMA gather, 8 PE transposes via identity,
    tap-major 3x4 fp32r matmuls, ACT bias, store.
Engine budget per core: PE ~26us (96 conv + 12 word matmuls), DVE ~22us
(32 max-reduces + 2 one-hot chunks), Pool ~17us (gather prep + 18
one-hots), ACT ~17us (10 one-hots + copies/bias).
"""
import os
import sys

for _p in ("/opt/trn_rl_repo", "/root/.axon_site/_ro/trn_rl_repo"):
    if os.path.isdir(_p) and _p not in sys.path:
        sys.path.insert(0, _p)

import numpy as np
import ml_dtypes
from contextlib import ExitStack

import concourse.bass as bass
import concourse.tile as tile
from concourse import bacc, mybir
from concourse.bass_utils import run_bass_kernel_spmd

B, L, C, D = 32, 256, 16, 128
WORD_VOCAB, CHR_VOCAB = 50000, 128
NCORES = 8
SPC = B // NCORES            # sentences per core (4)
WPC = SPC * L                # words per core (1024)
WPT = 32                     # words per char-tile
NT = WPC // WPT              # char tiles per core (32)
TILE_COLS = 546              # 1 lead pad + 32*17 (16 chars + pad per word) + 1
NJ = WPC // 128              # word-gather groups (8)
TPS = L // WPT               # tiles per sentence (8)

BF16 = ml_dtypes.bfloat16

LAST_EXEC_TIME_NS = None

_compiled = {}

# one-hot chunk layout: (start, ntiles, engine). Any 128-partition DMA
# costs ~2.6us of packet-issue latency per queue, so the first chunks are
# split across two queues by partition range and kept small.
OH_CHUNKS = [(0, 1, "dve"), (1, 1, "dve"), (2, 2, "dve"), (4, 4, "dve"),
             (8, 4, "act"), (12, 4, "act"), (16, 4, "dve"), (20, 4, "dve"),
             (24, 4, "act"), (28, 4, "dve")]


def _build_nc():
    nc = bacc.Bacc("TRN2", target_bir_lowering=False, debug=False,
                   num_devices=NCORES)
    f32, f32r, i32 = mybir.dt.float32, mybir.dt.float32r, mybir.dt.int32
    bf16 = mybir.dt.bfloat16

    t_cidx = nc.dram_tensor("cidx", [1, NT * TILE_COLS], bf16, kind="ExternalInput").ap()
    t_widx = nc.dram_tensor("widx", [128, NJ], i32, kind="ExternalInput").ap()
    t_wtab = nc.dram_tensor("wtab", [WORD_VOCAB, D], f32, kind="ExternalInput").ap()
    t_utab = nc.dram_tensor("utab", [128, 3 * D], bf16, kind="ExternalInput").ap()
    t_www = nc.dram_tensor("www", [D, 3, D], f32r, kind="ExternalInput").ap()
    t_call = nc.dram_tensor("call", [D, 133], f32, kind="ExternalInput").ap()

    o_ow = nc.dram_tensor("ow", [SPC, D, L], f32, kind="ExternalOutput").ap()
    o_oc = nc.dram_tensor("oc", [SPC, D, L], f32, kind="ExternalOutput").ap()

    with tile.TileContext(nc) as tc, ExitStack() as ctx:
        consts = ctx.enter_context(tc.tile_pool(name="consts", bufs=1))
        bigp = ctx.enter_context(tc.tile_pool(name="bigp", bufs=1))
        oh_d = ctx.enter_context(tc.tile_pool(name="oh_d", bufs=3))
        oh_a = ctx.enter_context(tc.tile_pool(name="oh_a", bufs=3))
        t1_a = ctx.enter_context(tc.tile_pool(name="t1_a", bufs=2))
        ps_y = ctx.enter_context(tc.tile_pool(name="ps_y", bufs=4, space="PSUM"))
        ps_w = ctx.enter_context(tc.tile_pool(name="ps_w", bufs=4, space="PSUM"))

        # ---- constants: partition-sliced across three queues to beat the
        # ~20ns/packet per-queue DMA issue rate ----
        s_call = consts.tile([D, 133], f32, tag="call")

        def call_slice(q, p0, p1):
            q.dma_start(
                out=s_call[p0:p1, :],
                in_=bass.AP(tensor=t_call.tensor, offset=p0 * 133,
                            ap=[[133, p1 - p0], [1, 133]]),
            )

        call_slice(nc.sync, 0, 43)
        call_slice(nc.scalar, 43, 86)
        call_slice(nc.gpsimd, 86, 128)
        s_niota = s_call[:, 0:1]
        s_onesc = s_call[:, 1:2]
        s_wb = s_call[:, 2:3]
        s_zero = s_call[:, 3:4]
        s_iotaf = s_call[:, 4:5]
        s_ident = s_call[:, 5:133]

        # ---- broadcasts; first chunks split by partition across queues ----
        s_bc = bigp.tile([128, NT * TILE_COLS], bf16, tag="bc")

        def issue_bc(ci, qs):
            lo, n, eng = OH_CHUNKS[ci]
            w = n * TILE_COLS
            nq = len(qs)
            step = 128 // nq
            for qi, q in enumerate(qs):
                p0, p1 = qi * step, (qi + 1) * step if qi < nq - 1 else 128
                q.dma_start(
                    out=s_bc[p0:p1, lo * TILE_COLS:lo * TILE_COLS + w],
                    in_=bass.AP(tensor=t_cidx.tensor, offset=lo * TILE_COLS,
                                ap=[[0, p1 - p0], [1, w]]),
                )

        issue_bc(0, [nc.sync, nc.scalar])
        issue_bc(1, [nc.sync, nc.scalar])
        issue_bc(2, [nc.sync, nc.scalar])
        s_ut = consts.tile([128, 3 * D], bf16, tag="utab")
        nc.sync.dma_start(s_ut[:], t_utab)
        issue_bc(3, [nc.scalar])
        issue_bc(4, [nc.sync])

        # ---- gpsimd queue: word indices + gathers ----
        s_widx = consts.tile([128, NJ], i32, tag="widx")
        nc.gpsimd.dma_start(s_widx[:], t_widx)
        s_wg = bigp.tile([128, NJ, D], f32, tag="wg")
        for j in range(NJ):
            nc.gpsimd.indirect_dma_start(
                out=s_wg[:, j, :], out_offset=None, in_=t_wtab,
                in_offset=bass.IndirectOffsetOnAxis(ap=s_widx[:, j:j + 1], axis=0),
            )

        issue_bc(5, [nc.scalar])
        issue_bc(6, [nc.sync])
        s_www = consts.tile([D, 3, D], f32r, tag="www")
        nc.scalar.dma_start(s_www[:], t_www)
        issue_bc(7, [nc.sync])
        issue_bc(8, [nc.scalar])
        issue_bc(9, [nc.sync])

        # ---- PE warm-up: zeros matmuls ramp the clock before real work ----
        s_zt = consts.tile([128, 512], bf16, tag="zt")
        nc.vector.memset(s_zt[:], 0.0)
        for i in range(8):
            pz = ps_w.tile([128, 512], f32, tag="ps_w", name=f"pz{i}")
            nc.tensor.matmul(pz[:], s_zt[:, 0:128], s_zt[:], start=True, stop=True)

        # ---- word-embedding transpose target / zero padding columns ----
        WEMB_COLS = SPC * (L + 1) + 1
        s_wembT = bigp.tile([128, WEMB_COLS], f32r, tag="wembT")
        s_wout = bigp.tile([128, WPC], f32, tag="wout")
        _wpad = s_wembT[:]
        nc.vector.tensor_copy(
            bass.AP(tensor=_wpad.tensor, offset=_wpad.offset, ap=[_wpad.ap[0], [257, 5]]),
            s_zero.to_broadcast([128, 5]),
        )

        # ---- char one-hots ----
        s_oh = {}

        def oh_chunk(ci):
            lo, n, eng = OH_CHUNKS[ci]
            w = n * TILE_COLS
            src = s_bc[:, lo * TILE_COLS:lo * TILE_COLS + w]
            if eng == "dve":
                o = oh_d.tile([128, w], bf16, tag="oh", name=f"oh{ci}")
                nc.vector.tensor_scalar(
                    out=o[:], in0=src, scalar1=s_iotaf[:, :1], scalar2=None,
                    op0=mybir.AluOpType.is_equal,
                )
            else:
                t1 = t1_a.tile([128, w], bf16, tag="t1", name=f"t1_{ci}")
                nc.scalar.activation(
                    out=t1[:], in_=src,
                    func=mybir.ActivationFunctionType.Abs,
                    bias=s_niota[:, :1], scale=1.0,
                )
                o = oh_a.tile([128, w], bf16, tag="oha", name=f"oha{ci}")
                nc.scalar.activation(
                    out=o[:], in_=t1[:],
                    func=mybir.ActivationFunctionType.Relu,
                    bias=s_onesc[:, :1], scale=-1.0,
                )
            for i in range(n):
                s_oh[lo + i] = o[:, i * TILE_COLS:(i + 1) * TILE_COLS]

        emit_at = {16: [6], 20: [7], 24: [8], 26: [9]}
        for ci in range(6):
            oh_chunk(ci)

        s_cf = bigp.tile([128, WPC], f32, tag="cf")

        def out_dma(s, half_split):
            lo = s * L
            if half_split:
                nc.sync.dma_start(
                    out=bass.AP(tensor=o_oc.tensor, offset=s * D * L,
                                ap=[[L, 64], [1, L]]),
                    in_=s_cf[0:64, lo:lo + L])
                nc.scalar.dma_start(
                    out=bass.AP(tensor=o_oc.tensor, offset=s * D * L + 64 * L,
                                ap=[[L, 64], [1, L]]),
                    in_=s_cf[64:128, lo:lo + L])
            else:
                nc.sync.dma_start(out=o_oc[s], in_=s_cf[:, lo:lo + L])

        def char_tile(t):
            for ci in emit_at.get(t, ()):
                oh_chunk(ci)
            a = s_oh[t]
            py = ps_y.tile([128, WPT, 16], f32, tag="ps_y")

            def ohs(off):
                return bass.AP(tensor=a.tensor, offset=a.offset + off,
                               ap=[a.ap[0], [17, WPT], [1, 16]])

            nc.tensor.matmul(py[:], s_ut[:, D:2 * D], ohs(1), start=True, stop=False)
            nc.tensor.matmul(py[:], s_ut[:, 0:D], ohs(0), start=False, stop=False)
            nc.tensor.matmul(py[:], s_ut[:, 2 * D:3 * D], ohs(2), start=False, stop=True)
            nc.vector.tensor_reduce(
                out=s_cf[:, t * WPT:(t + 1) * WPT], in_=py[:],
                axis=mybir.AxisListType.X, op=mybir.AluOpType.max,
            )
            if t % TPS == TPS - 1:
                out_dma(t // TPS, half_split=(t // TPS == SPC - 1))

        for t in range(14):
            char_tile(t)

        # ---- word path (fp32 family) ----
        for j in range(NJ):
            pt = ps_w.tile([128, 128], f32, tag="ps_w", name=f"pt{j}")
            nc.tensor.transpose(pt[:], s_wg[:, j, :], s_ident)
            base = 257 * (j // 2) + 1 + (j % 2) * 128
            nc.scalar.activation(out=s_wembT[:, base:base + 128], in_=pt[:],
                                 func=mybir.ActivationFunctionType.Copy)
        pwb = [ps_w.tile([128, L], f32, tag="ps_w", name=f"pwb{i}") for i in range(SPC)]
        pw = [pwb[s][:] for s in range(SPC)]
        for k, start, stop in ((1, True, False), (0, False, False), (2, False, True)):
            for s in range(SPC):
                base = 257 * s + k
                nc.tensor.matmul(pw[s], s_www[:, k, :],
                                 s_wembT[:, base:base + L], start=start, stop=stop)
        for s in range(SPC):
            nc.vector.tensor_scalar(
                out=s_wout[:, s * L:(s + 1) * L], in0=pw[s],
                scalar1=s_wb[:, :1], scalar2=None, op0=mybir.AluOpType.add,
            )
            nc.sync.dma_start(out=o_ow[s], in_=s_wout[:, s * L:(s + 1) * L])

        # ---- remaining char tiles ----
        for t in range(14, NT):
            char_tile(t)

    nc.compile()
    return nc


def _get_nc():
    if "nc" not in _compiled:
        _compiled["nc"] = _build_nc()
    return _compiled["nc"]


def _host_prep(word_vector, words_in_char):
    """Per-core index layouts (pure relayout/cast of the integer inputs)."""
    wv = np.asarray(word_vector).astype(np.int32).reshape(NCORES, WPC)
    wc = np.asarray(words_in_char).astype(np.int32).reshape(NCORES, NT, WPT, C)

    # padded char index rows: per tile of 32 words, period-17 layout,
    # -1 separators (one-hot of -1 is all-zero = conv zero padding)
    blocks = np.full((NCORES, NT, WPT, 17), -1.0, dtype=np.float32)
    blocks[..., :16] = wc
    lead = np.full((NCORES, NT, 1), -1.0, dtype=np.float32)
    cidx = np.concatenate(
        [lead, blocks.reshape(NCORES, NT, WPT * 17), lead], axis=2
    ).reshape(NCORES, 1, NT * TILE_COLS).astype(BF16)

    # word indices wrapped for the fused 128x8 indirect gather
    widx = wv.reshape(NCORES, NJ, 128).transpose(0, 2, 1).copy()
    return cidx, widx


def kernel(**inputs):
    global LAST_EXEC_TIME_NS
    wt = np.ascontiguousarray(np.asarray(inputs["word_table"], dtype=np.float32))
    ct = np.asarray(inputs["chr_table"], dtype=np.float32)
    ccw = np.asarray(inputs["conv_chr_w"], dtype=np.float32)
    ccb = np.asarray(inputs["conv_chr_b"], dtype=np.float32)
    cww = np.asarray(inputs["conv_word_w"], dtype=np.float32)
    cwb = np.asarray(inputs["conv_word_b"], dtype=np.float32)

    cidx, widx = _host_prep(inputs["word_vector"], inputs["words_in_char"])

    # UT_k = chr_table @ W_k.T  [vocab=128, d_out=128]; char bias folded
    # into the tap-1 table (bias commutes with the max over positions).
    # ccw is [D_out, D_in, 3]: ut[v, k, o] = sum_d ct[v, d] * ccw[o, d, k]
    ut = np.einsum("vd,odk->vko", ct, ccw)
    ut[:, 1, :] += ccb[None, :]
    utab = np.ascontiguousarray(ut.reshape(128, 3 * D)).astype(BF16)

    call = np.zeros((D, 133), dtype=np.float32)
    call[:, 0] = -np.arange(128, dtype=np.float32)
    call[:, 1] = 1.0
    call[:, 2] = cwb
    call[:, 3] = 0.0
    call[:, 4] = np.arange(128, dtype=np.float32)
    call[:, 5:133] = np.eye(128, dtype=np.float32)

    shared = {
        "wtab": wt,
        "utab": utab,
        "www": np.ascontiguousarray(cww.transpose(1, 2, 0)),
        "call": call,
    }
    in_maps = [
        dict(shared, cidx=cidx[c], widx=widx[c]) for c in range(NCORES)
    ]

    nc = _get_nc()
    res = run_bass_kernel_spmd(nc, in_maps, core_ids=list(range(NCORES)))
    LAST_EXEC_TIME_NS = res.exec_time_ns
    globals()["LAST_RESULT"] = res

    full = np.empty((2, B, D, L), dtype=np.float32)
    for c in range(NCORES):
        full[0, c * SPC:(c + 1) * SPC] = res.results[c]["ow"]
        full[1, c * SPC:(c + 1) * SPC] = res.results[c]["oc"]
    return full


if __name__ == "__main__":
    rng = np.random.default_rng(0)
    ins = dict(
        word_vector=rng.integers(0, WORD_VOCAB, size=(B, L)).astype(np.int64),
        words_in_char=rng.integers(0, CHR_VOCAB, size=(B, L, C)).astype(np.int64),
        word_table=rng.standard_normal((WORD_VOCAB, D), dtype=np.float32) * 0.02,
        chr_table=rng.standard_normal((CHR_VOCAB, D), dtype=np.float32) * 0.02,
        conv_chr_w=rng.standard_normal((D, D, 3), dtype=np.float32) * 0.05,
        conv_chr_b=rng.standard_normal((D,), dtype=np.float32) * 0.05,
        conv_word_w=rng.standard_normal((D, D, 3), dtype=np.float32) * 0.05,
        conv_word_b=rng.standard_normal((D,), dtype=np.float32) * 0.05,
    )
    ins["word_table"][0] = 0
    ins["chr_table"][0] = 0
    out = kernel(**ins)
    print("out shape:", out.shape, "exec_ns:", LAST_EXEC_TIME_NS)
